# revision 9
# baseline (speedup 1.0000x reference)
"""Trainium2 Bass kernel for nn_Discriminator (batched bilinear form).

scores[b] = features[b] . (summary[b] @ weight.T)   for b in [0, 131072)

Strategy: data-parallel over 8 NeuronCores (batch sharded, weight replicated).
Per core (16384 rows = 128 tiles of 128):
  - DMA summary/features tiles (natural [b,h] layout) in 1MiB blocks
  - PE transposes summary tile -> PSUM (fp32, exact)
  - ACT copies PSUM -> SBUF rounding to float32r (TF32-like, 11 mantissa bits)
  - PE: 4 accumulating fp32r matmuls (full rate): ws = S @ W^T  in PSUM
  - DVE: fused multiply+reduce (tensor_tensor_reduce): scores_col = sum(F * ws)
  - Epilogue: transpose the [p, tile] score accumulator and DMA out.
"""

import numpy as np

B = 131072
H = 512
NCORES = 8
BC = B // NCORES      # rows per core
P = 128               # partitions
T = BC // P           # batch tiles per core (128)
CHUNKS = H // P       # k-chunks (4)
NB = 8                # batch tiles per DMA block (8 -> 2MiB per dma_start)
BUFS_BLOCKS = 3       # block double-buffering depth
BUFS_PS = 2           # PSUM pool depth
BUFS_ST = 3
BUFS_SCR = 2

_CACHE = {}


def _build():
    from concourse import bacc
    import concourse.mybir as mybir
    import concourse.tile as tile

    dt = mybir.dt
    nc = bacc.Bacc("TRN2", target_bir_lowering=False)

    feat = nc.dram_tensor("features", [BC, H], dt.float32, kind="ExternalInput")
    summ = nc.dram_tensor("summary", [BC, H], dt.float32, kind="ExternalInput")
    wt = nc.dram_tensor("wt", [H, H], dt.float32, kind="ExternalInput")  # weight.T
    ident_in = nc.dram_tensor("ident", [P, P], dt.float32, kind="ExternalInput")
    scores = nc.dram_tensor("scores", [BC], dt.float32, kind="ExternalOutput")

    # DRAM views
    feat_v = feat.ap().rearrange("(n p) h -> p n h", p=P)   # [128, T, 512]
    summ_v = summ.ap().rearrange("(n p) h -> p n h", p=P)
    wt_v = wt.ap().rearrange("(c p) h -> p c h", p=P)       # [128, 4, 512]
    scores_v = scores.ap().rearrange("(t p) -> t p", p=P)   # [T, 128]

    with tile.TileContext(nc) as tc:
        from contextlib import ExitStack
        with ExitStack() as ctx:
            singles = ctx.enter_context(tc.tile_pool(name="singles", bufs=1))
            blocks = ctx.enter_context(tc.tile_pool(name="blocks", bufs=BUFS_BLOCKS))
            stp = ctx.enter_context(tc.tile_pool(name="stp", bufs=BUFS_ST))
            scr = ctx.enter_context(tc.tile_pool(name="scr", bufs=BUFS_SCR))
            psT = ctx.enter_context(tc.tile_pool(name="psT", bufs=BUFS_PS, space="PSUM"))
            psW = ctx.enter_context(tc.tile_pool(name="psW", bufs=BUFS_PS, space="PSUM"))

            ident = singles.tile([P, P], dt.float32)
            nc.sync.dma_start(out=ident[:], in_=ident_in[:])

            wt_sb = singles.tile([P, CHUNKS, H], dt.float32)
            nc.sync.dma_start(out=wt_sb[:], in_=wt_v[:])
            wt_r = singles.tile([P, CHUNKS, H], dt.float32r)
            nc.scalar.copy(wt_r[:], wt_sb[:])  # round once to fp32r

            scores_acc = singles.tile([P, T], dt.float32)

            s_blk = None
            f_blk = None
            for t in range(T):
                nb = t % NB
                if nb == 0:
                    s_blk = blocks.tile([P, NB, H], dt.float32, tag="s_blk")
                    f_blk = blocks.tile([P, NB, H], dt.float32, tag="f_blk")
                    nc.sync.dma_start(out=s_blk[:], in_=summ_v[:, t:t + NB, :])
                    nc.sync.dma_start(out=f_blk[:], in_=feat_v[:, t:t + NB, :])

                # transpose S tile: 4x [128,128] -> PSUM (fp32 exact)
                ps_t = psT.tile([P, CHUNKS, P], dt.float32)
                for c in range(CHUNKS):
                    nc.tensor.transpose(
                        ps_t[:, c, :],
                        s_blk[:, nb, c * P:(c + 1) * P],
                        ident[:],
                    )
                # round to fp32r while copying PSUM -> SBUF
                st_r = stp.tile([P, CHUNKS, P], dt.float32r)
                nc.scalar.copy(st_r[:], ps_t[:])

                # ws[b, h] = sum_k S[b, k] * W[h, k] : 4 accumulating matmuls
                ps_w = psW.tile([P, H], dt.float32)
                for c in range(CHUNKS):
                    nc.tensor.matmul(
                        ps_w[:],
                        st_r[:, c, :],
                        wt_r[:, c, :],
                        start=(c == 0),
                        stop=(c == CHUNKS - 1),
                    )

                # scores[:, t] = sum_h F * ws  (fused multiply+reduce on DVE)
                mm_scr = scr.tile([P, H], dt.float32)
                nc.vector.scalar_tensor_tensor(
                    out=mm_scr[:],
                    in0=f_blk[:, nb, :],
                    scalar=1.0,
                    in1=ps_w[:],
                    op0=mybir.AluOpType.mult,
                    op1=mybir.AluOpType.mult,
                    accum_out=scores_acc[:, t:t + 1],
                )

            # epilogue: scores_acc[p, t] -> scores[t*128 + p]
            ps_fin = psT.tile([T, P], dt.float32, tag="ps_fin")
            nc.tensor.transpose(ps_fin[:], scores_acc[:], ident[:])
            out_sb = singles.tile([T, P], dt.float32)
            nc.scalar.copy(out_sb[:], ps_fin[:])
            nc.sync.dma_start(out=scores_v[:], in_=out_sb[:])

    nc.finalize()
    return nc


def _get_nc():
    if "nc" not in _CACHE:
        _CACHE["nc"] = _build()
    return _CACHE["nc"]


def kernel(features, summary, weight):
    from concourse.bass_utils import run_bass_kernel_spmd

    features = np.ascontiguousarray(np.asarray(features, dtype=np.float32))
    summary = np.ascontiguousarray(np.asarray(summary, dtype=np.float32))
    weight = np.asarray(weight, dtype=np.float32)
    wt = np.ascontiguousarray(weight.T)

    ident = np.eye(P, dtype=np.float32)
    nc = _get_nc()
    in_maps = [
        {
            "features": features[i * BC:(i + 1) * BC],
            "summary": summary[i * BC:(i + 1) * BC],
            "wt": wt,
            "ident": ident,
        }
        for i in range(NCORES)
    ]
    res = run_bass_kernel_spmd(nc, in_maps, core_ids=list(range(NCORES)))
    return np.concatenate([r["scores"] for r in res.results])


if __name__ == "__main__":
    rng = np.random.default_rng(0)
    f = rng.standard_normal((B, H), dtype=np.float32)
    s = rng.standard_normal((B, H), dtype=np.float32)
    w = (rng.random((H, H), dtype=np.float32) - 0.5) * (2.0 / np.sqrt(H))
    got = kernel(f, s, w)
    want = ((s @ w.T) * f).sum(-1)
    err = np.abs(got - want)
    print("absmax-rel:", err.max() / np.abs(want).max())


# revision 11
# speedup vs baseline: 1.0446x; 1.0446x over previous
"""Trainium2 Bass kernel for nn_Discriminator (batched bilinear form).

scores[b] = features[b] . (summary[b] @ weight.T)   for b in [0, 131072)

Strategy: data-parallel over 8 NeuronCores (batch sharded, weight replicated).
Per core (16384 rows = 128 tiles of 128):
  - DMA summary/features tiles (natural [b,h] layout) in 1MiB blocks
  - PE transposes summary tile -> PSUM (fp32, exact)
  - ACT copies PSUM -> SBUF rounding to float32r (TF32-like, 11 mantissa bits)
  - PE: 4 accumulating fp32r matmuls (full rate): ws = S @ W^T  in PSUM
  - DVE: fused multiply+reduce (tensor_tensor_reduce): scores_col = sum(F * ws)
  - Epilogue: transpose the [p, tile] score accumulator and DMA out.
"""

import numpy as np

B = 131072
H = 512
NCORES = 8
BC = B // NCORES      # rows per core
P = 128               # partitions
T = BC // P           # batch tiles per core (128)
CHUNKS = H // P       # k-chunks (4)
NB = 8                # batch tiles per DMA block (8 -> 2MiB per dma_start)
BUFS_BLOCKS = 3       # block double-buffering depth
BUFS_PS = 2           # PSUM pool depth
BUFS_ST = 3
BUFS_SCR = 2

_CACHE = {}


def _build():
    from concourse import bacc
    import concourse.mybir as mybir
    import concourse.tile as tile

    dt = mybir.dt
    nc = bacc.Bacc("TRN2", target_bir_lowering=False)

    feat = nc.dram_tensor("features", [BC, H], dt.float32, kind="ExternalInput")
    summ = nc.dram_tensor("summary", [BC, H], dt.float32, kind="ExternalInput")
    wt = nc.dram_tensor("wt", [H, H], dt.float32, kind="ExternalInput")  # weight.T
    ident_in = nc.dram_tensor("ident", [P, P], dt.float32, kind="ExternalInput")
    scores = nc.dram_tensor("scores", [BC], dt.float32, kind="ExternalOutput")

    # DRAM views
    feat_v = feat.ap().rearrange("(n p) h -> p n h", p=P)   # [128, T, 512]
    summ_v = summ.ap().rearrange("(n p) h -> p n h", p=P)
    wt_v = wt.ap().rearrange("(c p) h -> p c h", p=P)       # [128, 4, 512]
    scores_v = scores.ap().rearrange("(t p) -> t p", p=P)   # [T, 128]

    with tile.TileContext(nc) as tc:
        from contextlib import ExitStack
        with ExitStack() as ctx:
            singles = ctx.enter_context(tc.tile_pool(name="singles", bufs=1))
            blocks = ctx.enter_context(tc.tile_pool(name="blocks", bufs=BUFS_BLOCKS))
            stp = ctx.enter_context(tc.tile_pool(name="stp", bufs=BUFS_ST))
            scr = ctx.enter_context(tc.tile_pool(name="scr", bufs=BUFS_SCR))
            psT = ctx.enter_context(tc.tile_pool(name="psT", bufs=BUFS_PS, space="PSUM"))
            psW = ctx.enter_context(tc.tile_pool(name="psW", bufs=BUFS_PS, space="PSUM"))
            psF = ctx.enter_context(tc.tile_pool(name="psF", bufs=1, space="PSUM"))

            ident = singles.tile([P, P], dt.float32)
            nc.sync.dma_start(out=ident[:], in_=ident_in[:])

            wt_sb = singles.tile([P, CHUNKS, H], dt.float32)
            nc.sync.dma_start(out=wt_sb[:], in_=wt_v[:])
            wt_r = singles.tile([P, CHUNKS, H], dt.float32r)
            nc.scalar.copy(wt_r[:], wt_sb[:])  # round once to fp32r

            scores_acc = singles.tile([P, T], dt.float32)

            s_blk = None
            f_blk = None
            for t in range(T):
                nb = t % NB
                if nb == 0:
                    s_blk = blocks.tile([P, NB, H], dt.float32, tag="s_blk")
                    f_blk = blocks.tile([P, NB, H], dt.float32, tag="f_blk")
                    nc.sync.dma_start(out=s_blk[:], in_=summ_v[:, t:t + NB, :])
                    nc.sync.dma_start(out=f_blk[:], in_=feat_v[:, t:t + NB, :])

                # transpose S tile: 4x [128,128] -> PSUM (fp32 exact)
                ps_t = psT.tile([P, CHUNKS, P], dt.float32)
                for c in range(CHUNKS):
                    nc.tensor.transpose(
                        ps_t[:, c, :],
                        s_blk[:, nb, c * P:(c + 1) * P],
                        ident[:],
                    )
                # round to fp32r while copying PSUM -> SBUF
                st_r = stp.tile([P, CHUNKS, P], dt.float32r)
                nc.scalar.copy(st_r[:], ps_t[:])

                # ws[b, h] = sum_k S[b, k] * W[h, k] : 4 accumulating matmuls
                ps_w = psW.tile([P, H], dt.float32)
                for c in range(CHUNKS):
                    nc.tensor.matmul(
                        ps_w[:],
                        st_r[:, c, :],
                        wt_r[:, c, :],
                        start=(c == 0),
                        stop=(c == CHUNKS - 1),
                    )

                # scores[:, t] = sum_h F * ws  (fused multiply+reduce on DVE)
                mm_scr = scr.tile([P, H], dt.float32)
                nc.vector.scalar_tensor_tensor(
                    out=mm_scr[:],
                    in0=f_blk[:, nb, :],
                    scalar=1.0,
                    in1=ps_w[:],
                    op0=mybir.AluOpType.mult,
                    op1=mybir.AluOpType.mult,
                    accum_out=scores_acc[:, t:t + 1],
                )

            # epilogue: scores_acc[p, t] -> scores[t*128 + p]
            ps_fin = psF.tile([T, P], dt.float32)
            nc.tensor.transpose(ps_fin[:], scores_acc[:], ident[:])
            out_sb = singles.tile([T, P], dt.float32)
            nc.scalar.copy(out_sb[:], ps_fin[:])
            nc.sync.dma_start(out=scores_v[:], in_=out_sb[:])

    nc.finalize()
    return nc


def _get_nc():
    if "nc" not in _CACHE:
        _CACHE["nc"] = _build()
    return _CACHE["nc"]


def kernel(features, summary, weight):
    from concourse.bass_utils import run_bass_kernel_spmd

    features = np.ascontiguousarray(np.asarray(features, dtype=np.float32))
    summary = np.ascontiguousarray(np.asarray(summary, dtype=np.float32))
    weight = np.asarray(weight, dtype=np.float32)
    wt = np.ascontiguousarray(weight.T)

    ident = np.eye(P, dtype=np.float32)
    nc = _get_nc()
    in_maps = [
        {
            "features": features[i * BC:(i + 1) * BC],
            "summary": summary[i * BC:(i + 1) * BC],
            "wt": wt,
            "ident": ident,
        }
        for i in range(NCORES)
    ]
    res = run_bass_kernel_spmd(nc, in_maps, core_ids=list(range(NCORES)))
    return np.concatenate([r["scores"] for r in res.results])


if __name__ == "__main__":
    rng = np.random.default_rng(0)
    f = rng.standard_normal((B, H), dtype=np.float32)
    s = rng.standard_normal((B, H), dtype=np.float32)
    w = (rng.random((H, H), dtype=np.float32) - 0.5) * (2.0 / np.sqrt(H))
    got = kernel(f, s, w)
    want = ((s @ w.T) * f).sum(-1)
    err = np.abs(got - want)
    print("absmax-rel:", err.max() / np.abs(want).max())


# revision 18
# speedup vs baseline: 1.0965x; 1.0497x over previous
"""Trainium2 Bass kernel for nn_Discriminator (batched bilinear form).

scores[b] = features[b] . (summary[b] @ weight.T)   for b in [0, 131072)

Strategy: data-parallel over 8 NeuronCores (batch sharded, weight replicated).
Per core (16384 rows = 128 tiles of 128):
  - DMA summary/features tiles (natural [b,h] layout) in 1MiB blocks
  - PE transposes summary tile -> PSUM (fp32, exact)
  - ACT copies PSUM -> SBUF rounding to float32r (TF32-like, 11 mantissa bits)
  - PE: 4 accumulating fp32r matmuls (full rate): ws = S @ W^T  in PSUM
  - DVE: fused multiply+reduce (tensor_tensor_reduce): scores_col = sum(F * ws)
  - Epilogue: transpose the [p, tile] score accumulator and DMA out.
"""

import numpy as np

B = 131072
H = 512
NCORES = 8
BC = B // NCORES      # rows per core
P = 128               # partitions
T = BC // P           # batch tiles per core (128)
CHUNKS = H // P       # k-chunks (4)
NB = 8                # batch tiles per DMA block (8 -> 2MiB per dma_start)
BUFS_BLOCKS = 3       # block double-buffering depth
BUFS_PS = 2           # PSUM pool depth
BUFS_ST = 3
BUFS_SCR = 2
OUT_GROUP = 32        # tiles per streamed score-output group

_CACHE = {}


def _build():
    from concourse import bacc
    import concourse.mybir as mybir
    import concourse.tile as tile

    dt = mybir.dt
    nc = bacc.Bacc("TRN2", target_bir_lowering=False)

    feat = nc.dram_tensor("features", [BC, H], dt.float32, kind="ExternalInput")
    summ = nc.dram_tensor("summary", [BC, H], dt.float32, kind="ExternalInput")
    wt = nc.dram_tensor("wt", [H, H], dt.float32, kind="ExternalInput")  # weight.T
    ident_in = nc.dram_tensor("ident", [P, P], dt.float32, kind="ExternalInput")
    scores = nc.dram_tensor("scores", [BC], dt.float32, kind="ExternalOutput")

    # DRAM views
    feat_v = feat.ap().rearrange("(n p) h -> p n h", p=P)   # [128, T, 512]
    summ_v = summ.ap().rearrange("(n p) h -> p n h", p=P)
    wt_v = wt.ap().rearrange("(c p) h -> p c h", p=P)       # [128, 4, 512]
    scores_v = scores.ap().rearrange("(t p) -> t p", p=P)   # [T, 128]

    with tile.TileContext(nc) as tc:
        from contextlib import ExitStack
        with ExitStack() as ctx:
            singles = ctx.enter_context(tc.tile_pool(name="singles", bufs=1))
            blocks = ctx.enter_context(tc.tile_pool(name="blocks", bufs=BUFS_BLOCKS))
            stp = ctx.enter_context(tc.tile_pool(name="stp", bufs=BUFS_ST))
            scr = ctx.enter_context(tc.tile_pool(name="scr", bufs=BUFS_SCR))
            psT = ctx.enter_context(tc.tile_pool(name="psT", bufs=BUFS_PS, space="PSUM"))
            psW = ctx.enter_context(tc.tile_pool(name="psW", bufs=BUFS_PS, space="PSUM"))


            # first data blocks go first on the DMA ring so compute can
            # start as early as possible; wt/ident follow.
            s_blk = blocks.tile([P, NB, H], dt.float32, tag="s_blk")
            f_blk = blocks.tile([P, NB, H], dt.float32, tag="f_blk")
            nc.sync.dma_start(out=s_blk[:], in_=summ_v[:, 0:NB, :])
            nc.sync.dma_start(out=f_blk[:], in_=feat_v[:, 0:NB, :])

            ident = singles.tile([P, P], dt.float32)
            nc.sync.dma_start(out=ident[:], in_=ident_in[:])

            wt_sb = singles.tile([P, CHUNKS, H], dt.float32)
            nc.sync.dma_start(out=wt_sb[:], in_=wt_v[:])
            wt_r = singles.tile([P, CHUNKS, H], dt.float32r)
            nc.scalar.copy(wt_r[:], wt_sb[:])  # round once to fp32r

            og_size = min(OUT_GROUP, T)
            OG = T // og_size  # score output groups
            scores_accs = [
                singles.tile([P, og_size], dt.float32,
                             name=f"sacc{g}", tag=f"sacc{g}")
                for g in range(OG)
            ]

            G = T // 2  # tile pairs
            for g in range(G):
                t = 2 * g
                nb = t % NB
                if nb == 0 and t > 0:
                    s_blk = blocks.tile([P, NB, H], dt.float32, tag="s_blk")
                    f_blk = blocks.tile([P, NB, H], dt.float32, tag="f_blk")
                    nc.sync.dma_start(out=s_blk[:], in_=summ_v[:, t:t + NB, :])
                    nc.sync.dma_start(out=f_blk[:], in_=feat_v[:, t:t + NB, :])

                # transpose 2 S tiles: 8x [128,128] -> PSUM (fp32 exact)
                ps_t = psT.tile([P, 2, CHUNKS, P], dt.float32)
                for u in range(2):
                    for c in range(CHUNKS):
                        nc.tensor.transpose(
                            ps_t[:, u, c, :],
                            s_blk[:, nb + u, c * P:(c + 1) * P],
                            ident[:],
                        )
                # round to fp32r while copying PSUM -> SBUF (one copy / 2 tiles)
                st_r = stp.tile([P, 2, CHUNKS, P], dt.float32r)
                nc.scalar.copy(st_r[:], ps_t[:])

                # ws[b, h] = sum_k S[b, k] * W[h, k] : 4 accumulating matmuls/tile
                ps_w = psW.tile([P, 2, H], dt.float32)
                for u in range(2):
                    for c in range(CHUNKS):
                        nc.tensor.matmul(
                            ps_w[:, u, :],
                            st_r[:, u, c, :],
                            wt_r[:, c, :],
                            start=(c == 0),
                            stop=(c == CHUNKS - 1),
                        )

                # scores[:, t] = sum_h F * ws  (fused multiply+reduce on DVE)
                for u in range(2):
                    mm_scr = scr.tile([P, H], dt.float32)
                    nc.vector.scalar_tensor_tensor(
                        out=mm_scr[:],
                        in0=f_blk[:, nb + u, :],
                        scalar=1.0,
                        in1=ps_w[:, u, :],
                        op0=mybir.AluOpType.mult,
                        op1=mybir.AluOpType.mult,
                        accum_out=scores_accs[(t + u) // og_size]
                        [:, (t + u) % og_size:(t + u) % og_size + 1],
                    )

                # stream completed score groups out:
                # scores_acc[p, tl] -> scores[(g0+tl)*128 + p]
                if (t + 2) % og_size == 0:
                    og = (t + 2) // og_size - 1
                    ps_fin = psW.tile([og_size, P], dt.float32, tag="ps_w")
                    nc.tensor.transpose(ps_fin[:], scores_accs[og][:], ident[:])
                    out_sb = scr.tile([og_size, P], dt.float32, tag="out_sb")
                    nc.scalar.copy(out_sb[:], ps_fin[:])
                    nc.sync.dma_start(
                        out=scores_v[og * og_size:(og + 1) * og_size, :],
                        in_=out_sb[:],
                    )

    nc.finalize()
    return nc


def _get_nc():
    if "nc" not in _CACHE:
        _CACHE["nc"] = _build()
    return _CACHE["nc"]


def kernel(features, summary, weight):
    from concourse.bass_utils import run_bass_kernel_spmd

    features = np.ascontiguousarray(np.asarray(features, dtype=np.float32))
    summary = np.ascontiguousarray(np.asarray(summary, dtype=np.float32))
    weight = np.asarray(weight, dtype=np.float32)
    wt = np.ascontiguousarray(weight.T)

    ident = np.eye(P, dtype=np.float32)
    nc = _get_nc()
    in_maps = [
        {
            "features": features[i * BC:(i + 1) * BC],
            "summary": summary[i * BC:(i + 1) * BC],
            "wt": wt,
            "ident": ident,
        }
        for i in range(NCORES)
    ]
    res = run_bass_kernel_spmd(nc, in_maps, core_ids=list(range(NCORES)))
    return np.concatenate([r["scores"] for r in res.results])


if __name__ == "__main__":
    rng = np.random.default_rng(0)
    f = rng.standard_normal((B, H), dtype=np.float32)
    s = rng.standard_normal((B, H), dtype=np.float32)
    w = (rng.random((H, H), dtype=np.float32) - 0.5) * (2.0 / np.sqrt(H))
    got = kernel(f, s, w)
    want = ((s @ w.T) * f).sum(-1)
    err = np.abs(got - want)
    print("absmax-rel:", err.max() / np.abs(want).max())
